# revision 1
# baseline (speedup 1.0000x reference)
"""Distributed Trainium2 (8 NeuronCores) kernel for masked multi-head attention
+ output projection (nn_Attention_60790967107825).

Head-parallel attention, row-parallel projection, one AllToAll between:
  - Each core owns 2 of the 16 heads (all 4 batches) -> 8 (b,h) pairs/core.
  - Host prep: q/k fed pre-transposed per head (so the TensorE contraction
    needs no on-device transposes), everything cast to bf16, and keys
    COMPACTED per batch to the unmasked set (masked keys contribute
    exp(-inf)=0 exactly), padded to a multiple of 128; pad slots carry
    k=0 and a 0 in the ones-column appended to V, so they add 0 to both
    the PV numerator and the softmax denominator. ~2x less exp/matmul work
    at ~50% mask density. The ones-column also yields the denominators as
    row 64 of the PV accumulation for free.
  - Scores are computed transposed (S^T[j,i] = K Q^T) so P^T feeds the PV
    matmul directly as the moving operand with V stationary.
  - Numerators are scaled by m_i/Z_i (DVE fast reciprocal + DMA partition
    broadcast); the masked-query uniform-attention term is rank-1 per batch
    and is re-added after the projection from a V-mean column carried
    through the collective (u[r] * (Vmean_b @ W^T) + b_out).
  - Two AllToAlls (~1.1MB/rank each), one per head-half: the first issues
    halfway through attention and hides completely; PE warm-up matmuls
    bridge the second so the projection runs at full HAM clock.
  - After the exchange every core holds all 1024 channels for its own 1024
    output rows at identical local addresses (SPMD-clean) and runs the
    full projection locally; outputs concatenate on the host.
"""

import os
import sys

import numpy as np

for _p in ("/opt/trn_rl_repo", "/root/.axon_site/_ro/trn_rl_repo"):
    if os.path.isdir(_p) and _p not in sys.path:
        sys.path.insert(0, _p)

import ml_dtypes  # noqa: E402
import concourse.bass as bass  # noqa: E402,F401
import concourse.mybir as mybir  # noqa: E402
import concourse.tile as tile  # noqa: E402
from concourse import bacc  # noqa: E402
from concourse.bass_utils import run_bass_kernel_spmd  # noqa: E402

B, H, N, D = 4, 16, 2048, 64
DIM = H * D
P = 128
NCORES = 8
HPC = H // NCORES          # heads per core
PAIRS = B * HPC            # (b, h_local) pairs per core
SCALE = float(D) ** -0.5
IC = 2                     # query chunks per pair
ICW = N // IC              # 1024
RB = B * N // NCORES       # 1024 output rows per core
RBW = RB + 16              # a2a row width (col RB carries the V-mean)
CT = DIM // P              # 8 contraction tiles in the projection
MBIG = 1.0e30              # Z multiplier for masked queries -> 1/Z == 0

bf16 = mybir.dt.bfloat16
f32 = mybir.dt.float32
npbf = ml_dtypes.bfloat16

_CACHE = {}



def build_graph(nkb=(N,) * B):
    nk = max(nkb)
    nc = bacc.Bacc("TRN2", num_devices=NCORES)

    qT = nc.dram_tensor("qT", [PAIRS, D, N], bf16, kind="ExternalInput")
    kT = nc.dram_tensor("kT", [PAIRS, D, nk], bf16, kind="ExternalInput")
    vv = nc.dram_tensor("v", [PAIRS, nk, D + 1], bf16, kind="ExternalInput")
    minvD = nc.dram_tensor("minv", [PAIRS, N], f32, kind="ExternalInput")
    uD = nc.dram_tensor("uproj", [P, RB // P], f32, kind="ExternalInput")
    wTD = nc.dram_tensor("wT", [DIM, DIM], bf16, kind="ExternalInput")
    boutD = nc.dram_tensor("bout", [1, DIM], f32, kind="ExternalInput")
    vmD = nc.dram_tensor("vmean", [PAIRS, D], bf16, kind="ExternalInput")
    outD = nc.dram_tensor("out", [RB, DIM], f32, kind="ExternalOutput")

    with tile.TileContext(nc, num_cores=NCORES) as tc:
        with tc.tile_pool(name="dram", bufs=1, space="DRAM") as dramp:
            # one buffer pair per head-half: A2A #0 (h_local=0 heads) issues
            # after half the pairs and hides under the remaining compute
            a2a_in = [
                dramp.tile([NCORES, D, RBW], bf16, name=f"a2a_in{h}")
                for h in range(HPC)
            ]
            a2a_out = [
                dramp.tile([NCORES, D, RBW], bf16, name=f"a2a_out{h}")
                for h in range(HPC)
            ]
            zrow_dram = dramp.tile([PAIRS, N], bf16, name="zrow_dram")
            pvm_dram = dramp.tile([1, DIM], f32, name="pvm_dram")

            with tc.tile_pool(name="constp", bufs=1) as constp:
                wt_sb = constp.tile([P, CT, DIM], bf16, name="wt_sb")
                u_sb = constp.tile([P, RB // P], f32, name="u_sb")
                bout128 = constp.tile([P, DIM], f32, name="bout128")
                gat = constp.tile([P, CT, RBW], bf16, name="gat")

                def prefetch_proj_consts():
                    # emitted after the first pair's loads so they do not
                    # crowd the DMA queues ahead of the critical path
                    for ct in range(CT):
                        nc.sync.dma_start(
                            wt_sb[:, ct, :], wTD[ct * P : (ct + 1) * P, :]
                        )
                    nc.sync.dma_start(u_sb[:], uD[:])
                    nc.sync.dma_start(
                        bout128[:], boutD[0:1, :].to_broadcast((P, DIM))
                    )
                    for vpr in range(PAIRS):
                        vb, vhl = divmod(vpr, HPC)
                        for ic in range(IC):
                            nc.sync.dma_start(
                                a2a_in[vhl][HPC * vb + ic, :, RB : RB + 1],
                                vmD[vpr : vpr + 1, :].rearrange("o d -> d o"),
                            )

                with (
                    tc.tile_pool(name="qkp", bufs=3) as qkp,
                    tc.tile_pool(name="vpool", bufs=3) as vp,
                    tc.tile_pool(name="ptp", bufs=3) as ptp,
                    tc.tile_pool(name="onump", bufs=2) as onp,
                    tc.tile_pool(name="smallp", bufs=2) as smallp,
                    tc.tile_pool(name="finp", bufs=2) as finp,
                    tc.tile_pool(name="psS", bufs=2, space="PSUM") as psS,
                    tc.tile_pool(name="psO", bufs=2, space="PSUM") as psO,
                ):
                    first = True
                    for hl in range(HPC):
                        for b in range(B):
                            pr = b * HPC + hl
                            nk_b = nkb[b]
                            jtk = nk_b // P
                            qt = qkp.tile([P, N], bf16, tag="qt", name=f"qt{pr}")
                            kt = qkp.tile([P, nk], bf16, tag="kt", name=f"kt{pr}")
                            nc.any.memset(qt[D:, :], 0.0)
                            nc.any.memset(kt[D:, :nk_b], 0.0)
                            # split loads: the first S matmul only needs the
                            # leading slices, so it can start sooner (finest
                            # for the very first pair, which gates startup)
                            ksplits = (
                                (0, P, 2 * P, 4 * P, nk_b // 2, nk_b)
                                if first
                                else (0, nk_b // 2, nk_b)
                            )
                            for lo2, hi2 in zip(ksplits[:-1], ksplits[1:]):
                                if lo2 < hi2:
                                    nc.sync.dma_start(
                                        kt[:D, lo2:hi2], kT[pr, :, lo2:hi2]
                                    )
                            qsplits = (0, 512, 1024, N // 2, N) if first else (
                                0, N // 4, N // 2, 3 * N // 4, N
                            )
                            for lo2, hi2 in zip(qsplits[:-1], qsplits[1:]):
                                if lo2 < hi2:
                                    nc.sync.dma_start(
                                        qt[:D, lo2:hi2], qT[pr, :, lo2:hi2]
                                    )
                            vt = vp.tile(
                                [P, jtk, D + 1], bf16, tag="vt", name=f"vt{pr}"
                            )
                            t2 = max(jtk // 2, 1)
                            for lo, hi in ((0, t2), (t2, jtk)):
                                if lo >= hi:
                                    continue
                                nc.sync.dma_start(
                                    vt[:, lo:hi, :],
                                    vv[pr, lo * P : hi * P, :]
                                    .rearrange("(t pp) d -> pp t d", pp=P),
                                )
                            minv_p = smallp.tile(
                                [1, N], f32, tag="minvp", name=f"mi{pr}"
                            )
                            nc.sync.dma_start(minv_p[:], minvD[pr : pr + 1, :])
                            if first:
                                prefetch_proj_consts()
                                first = False

                            for ic in range(IC):
                                i0 = ic * ICW
                                o_ps = psO.tile(
                                    [D + 1, ICW], f32, tag="ops", name=f"o{pr}_{ic}"
                                )
                                for jt in range(jtk):
                                    s_ps = psS.tile(
                                        [P, ICW],
                                        f32,
                                        tag="sps",
                                        name=f"s{pr}_{ic}_{jt}",
                                    )
                                    for n0 in range(0, ICW, 512):
                                        nc.tensor.matmul(
                                            s_ps[:, n0 : n0 + 512],
                                            lhsT=kt[:, jt * P : (jt + 1) * P],
                                            rhs=qt[:, i0 + n0 : i0 + n0 + 512],
                                            start=True,
                                            stop=True,
                                        )
                                    pt = ptp.tile(
                                        [P, ICW],
                                        bf16,
                                        tag="pt",
                                        name=f"p{pr}_{ic}_{jt}",
                                    )
                                    nc.scalar.activation(
                                        pt[:],
                                        s_ps[:],
                                        mybir.ActivationFunctionType.Exp,
                                        scale=SCALE,
                                    )
                                    for n0 in range(0, ICW, 512):
                                        last_pv = nc.tensor.matmul(
                                            o_ps[:, n0 : n0 + 512],
                                            lhsT=vt[:, jt, :],
                                            rhs=pt[:, n0 : n0 + 512],
                                            start=(jt == 0),
                                            stop=(jt == jtk - 1),
                                        )
                                # evacuate PSUM, then the per-chunk z path:
                                # zm = m_i / Z_i (masked queries -> 0), scale
                                # the numerators and ship this chunk at once
                                onum = onp.tile(
                                    [D, ICW], bf16, tag="onum", name=f"on{pr}_{ic}"
                                )
                                nc.vector.tensor_copy(onum[:], o_ps[:D, :])
                                zpair = smallp.tile(
                                    [1, ICW], f32, tag="zpair", name=f"zp{pr}_{ic}"
                                )
                                nc.vector.tensor_copy(zpair[:], o_ps[D : D + 1, :])
                                zq = smallp.tile(
                                    [1, ICW], f32, tag="zq", name=f"zq{pr}_{ic}"
                                )
                                nc.vector.tensor_tensor(
                                    zq[:],
                                    zpair[:],
                                    minv_p[0:1, i0 : i0 + ICW],
                                    mybir.AluOpType.mult,
                                )
                                zr = smallp.tile(
                                    [1, ICW], f32, tag="zr", name=f"zr{pr}_{ic}"
                                )
                                nc.vector.reciprocal_approx_fast(zr[:], zq[:])
                                zrb = smallp.tile(
                                    [1, ICW], bf16, tag="zrb", name=f"zb{pr}_{ic}"
                                )
                                nc.vector.tensor_copy(zrb[:], zr[:])
                                nc.sync.dma_start(
                                    zrow_dram[pr : pr + 1, i0 : i0 + ICW], zrb[:]
                                )
                                zm64 = finp.tile(
                                    [D, ICW], bf16, tag="zm64", name=f"zm{pr}_{ic}"
                                )
                                nc.sync.dma_start(
                                    zm64[:],
                                    zrow_dram[
                                        pr : pr + 1, i0 : i0 + ICW
                                    ].to_broadcast((D, ICW)),
                                )
                                fin = finp.tile(
                                    [D, ICW], bf16, tag="fin", name=f"fi{pr}_{ic}"
                                )
                                last_fin = nc.vector.tensor_tensor(
                                    fin[:], onum[:], zm64[:], mybir.AluOpType.mult
                                )
                                nc.sync.dma_start(
                                    a2a_in[hl][HPC * b + ic, :, 0:RB], fin[:]
                                )

                        # this head-half is complete on every core: exchange
                        # it (the hl=0 round is fully hidden under compute)
                        nc.gpsimd.collective_compute(
                            "AllToAll",
                            mybir.AluOpType.bypass,
                            replica_groups=[list(range(NCORES))],
                            ins=[a2a_in[hl].opt()],
                            outs=[a2a_out[hl].opt()],
                        )

                with (
                    tc.tile_pool(name="outp", bufs=3) as outp,
                    tc.tile_pool(name="smallq", bufs=1) as smallq,
                    tc.tile_pool(name="psP", bufs=2, space="PSUM") as psP,
                    tc.tile_pool(name="psPV", bufs=1, space="PSUM") as psPV,
                    tc.tile_pool(name="psWarm", bufs=1, space="PSUM") as psW,
                ):
                    for h in range(HPC):
                        for ct in range(CT):
                            nc.sync.dma_start(
                                gat[h * D : (h + 1) * D, ct, :],
                                a2a_out[h][ct],
                            )

                    def pin(mm, after, why):
                        tile.add_dep_helper(mm.ins, after.ins, sync=False, reason=why)
                        return mm

                    warm_ps = psW.tile([P, 512], f32, name="warm_ps")
                    last_warm = last_pv
                    NWARM, GRP = 80, 10
                    for wi in range(NWARM):
                        last_warm = pin(
                            nc.tensor.matmul(
                                warm_ps[:],
                                lhsT=wt_sb[:, 0, 0:128],
                                rhs=wt_sb[:, 1, 0:512],
                                start=(wi % GRP == 0),
                                stop=(wi % GRP == GRP - 1),
                            ),
                            last_pv,
                            "warmups bridge the A2A window",
                        )

                    pvm_ps = psPV.tile([1, DIM], f32, name="pvm_ps")
                    for ct in range(CT):
                        for n0 in range(0, DIM, 512):
                            pin(
                                nc.tensor.matmul(
                                    pvm_ps[:, n0 : n0 + 512],
                                    lhsT=gat[:, ct, RB : RB + 1],
                                    rhs=wt_sb[:, ct, n0 : n0 + 512],
                                    start=(ct == 0),
                                    stop=(ct == CT - 1),
                                ),
                                last_warm,
                                "keep warmups ahead in the PE stream",
                            )
                    pvm_row = smallq.tile([1, DIM], f32, name="pvm_row")
                    pin(
                        nc.vector.tensor_copy(pvm_row[:], pvm_ps[:]),
                        last_fin,
                        "projection DVE ops stay behind attention DVE",
                    )
                    nc.sync.dma_start(pvm_dram[:], pvm_row[:])
                    pvm128 = smallq.tile([P, DIM], f32, name="pvm128")
                    nc.sync.dma_start(
                        pvm128[:], pvm_dram[0:1, :].to_broadcast((P, DIM))
                    )

                    for rt in range(RB // P):
                        o_ps = psP.tile([P, DIM], f32, tag="prps", name=f"pr{rt}")
                        for ct in range(CT):
                            for n0 in range(0, DIM, 512):
                                pin(
                                    nc.tensor.matmul(
                                        o_ps[:, n0 : n0 + 512],
                                        lhsT=gat[:, ct, rt * P : (rt + 1) * P],
                                        rhs=wt_sb[:, ct, n0 : n0 + 512],
                                        start=(ct == 0),
                                        stop=(ct == CT - 1),
                                    ),
                                    last_warm,
                                    "keep warmups ahead in the PE stream",
                                )
                        t1 = outp.tile([P, DIM], f32, tag="t1", name=f"t1_{rt}")
                        t1_op = nc.vector.scalar_tensor_tensor(
                            t1[:],
                            in0=pvm128[:],
                            scalar=u_sb[:, rt : rt + 1],
                            in1=bout128[:],
                            op0=mybir.AluOpType.mult,
                            op1=mybir.AluOpType.add,
                        )
                        pin(t1_op, last_fin, "projection DVE stays behind attention")
                        osb = outp.tile([P, DIM], f32, tag="osb", name=f"ob{rt}")
                        nc.vector.tensor_tensor(
                            osb[:], o_ps[:], t1[:], mybir.AluOpType.add
                        )
                        nc.sync.dma_start(outD[rt * P : (rt + 1) * P, :], osb[:])

    nc.compile()
    return nc


def _get_nc(nkb=(N,) * B):
    key = f"nc{nkb}"
    if key not in _CACHE:
        _CACHE[key] = build_graph(nkb)
    return _CACHE[key]


def key_budget(mask):
    """Per-batch compacted key counts (unmasked incl. CLS), padded to 128."""
    counts = 1 + np.asarray(mask).astype(bool).sum(axis=1)
    return tuple(
        min(max(int(-(-int(c) // P) * P), P), N) for c in counts
    )


def make_in_maps(q, k, v, mask, W_out, b_out, nkb=(N,) * B):
    nk = max(nkb)
    q16 = q.astype(npbf)
    k16 = k.astype(npbf)
    v16 = v.astype(npbf)
    m_full = np.concatenate(
        [np.ones((B, 1), dtype=bool), np.asarray(mask).astype(bool)], axis=1
    )  # [B, N]
    # key compaction: keep only unmasked keys (masked ones contribute
    # exp(-inf)=0 exactly); pad each batch to nk with bias-killed slots
    kC = np.zeros((B, H, nk, D), dtype=npbf)
    vC = np.zeros((B, H, nk, D + 1), dtype=npbf)
    for b in range(B):
        idx = np.flatnonzero(m_full[b])
        kC[b, :, : len(idx)] = k16[b][:, idx, :]
        vC[b, :, : len(idx), :D] = v16[b][:, idx, :]
        vC[b, :, : len(idx), D] = 1.0
    minv = np.where(m_full, 1.0, MBIG).astype(np.float32)[
        np.repeat(np.arange(B), HPC)
    ]  # [PAIRS, N]
    wT16 = np.ascontiguousarray(np.asarray(W_out).T).astype(npbf)
    bout = np.asarray(b_out).astype(np.float32).reshape(1, DIM)

    in_maps = []
    for c in range(NCORES):
        heads = slice(HPC * c, HPC * (c + 1))
        qTc = np.ascontiguousarray(
            q16[:, heads].transpose(0, 1, 3, 2).reshape(PAIRS, D, N)
        )
        kTc = np.ascontiguousarray(
            kC[:, heads].transpose(0, 1, 3, 2).reshape(PAIRS, D, nk)
        )
        vc = np.ascontiguousarray(vC[:, heads].reshape(PAIRS, nk, D + 1))
        vmc = (
            v16[:, heads].astype(np.float32).sum(axis=2).reshape(PAIRS, D)
        ).astype(npbf)
        bc = c // 2
        i0c = (c % 2) * RB
        u_core = np.ascontiguousarray(
            ((1.0 - m_full[bc, i0c : i0c + RB].astype(np.float32)) / N)
            .reshape(RB // P, P)
            .T
        )
        in_maps.append(
            {
                "qT": qTc,
                "kT": kTc,
                "v": vc,
                "minv": minv,
                "uproj": u_core,
                "wT": wT16,
                "vmean": vmc,
                "bout": bout,
            }
        )
    return in_maps


def run(q, k, v, mask, W_out, b_out, trace=False, **spmd_kwargs):
    nkb = key_budget(mask)
    nc = _get_nc(nkb)
    in_maps = make_in_maps(q, k, v, mask, W_out, b_out, nkb=nkb)
    res = run_bass_kernel_spmd(
        nc, in_maps, core_ids=list(range(NCORES)), trace=trace, **spmd_kwargs
    )
    outs = [np.asarray(res.results[c]["out"]) for c in range(NCORES)]
    full = np.concatenate(outs, axis=0).reshape(B, N, DIM).astype(np.float32)
    return full, res


def kernel(q, k, v, mask, W_out, b_out):
    out, _ = run(q, k, v, mask, W_out, b_out, trace=False)
    return out

